# revision 118
# baseline (speedup 1.0000x reference)
"""Trainium2 Bass kernel for nn_BucketedGoWatti (sparse windowed attention).

Algorithm (mathematically identical to the reference): the 19 overlapping
windows (stride 384, win 1536) are runs of 12 consecutive 128-row chunks, so
per (batch, L-half) core: A1/A2 = [8*Wk_core | 4*W2]^T H^T, S^T = A1^T qct,
X = exp(S), HV^T = A2^T G^T; per-window column sums of X and X*HV (window-
membership matmuls straight into window space) give softmax denominators E_w
and logits lw_w; Gamma_c = sum_{w contains c} exp(lw_w)/E_w, and
z = (X*Gamma)^T H in one output GEMM.  Host merges the two L-halves.

Performance structure (v3):
  1. Host-prepped layouts: H pre-cast/pre-transposed fp8 main+residual pairs;
     weights packed partition-major ([128, 16*256]) so every DMA line is
     >=512B (full DMA bandwidth); DMA order wk -> ht block0 -> w2 -> ht
     stream -> hn8 -> hnr(split) so the first A matmul issues at ~4.5us.
  2. fp8 DoubleRow matmuls; softmax-weight chains (A1, S, z) run 3 residual-
     compensated passes, logit-only chains (A2, HV) run 1 pass.
  3. Both window column sums run as fp8 DR pairs (ss from X8=X/16 cast on
     the idle Pool engine, dd from X*HV/256), lagged 3 pairs behind PH1 so
     Pool casts and early-Gamma reads never stall the PE.
  4. Early Gamma: windows 0/1 close at pair 7, so their rec/lw/exp/gam
     chain, the chunk 0-5 Gamma broadcasts (gpsimd partition ops, no DMA)
     and pr0's pp/pp8 all run mid-A-phase; the z GEMM starts right at the
     X-flush with no serial PH2 chain in front.
  5. z accumulators tt0-2 take the 6 PSUM banks freed by the A/X pools;
     Gamma slab matmuls reuse spare 32-aligned rows of the ss/dd banks and
     interleave with the first z pairs; tt3 reuses the ss/dd banks after
     the last slab (catch-up pass), so all 8 banks stay busy with zero
     pool-release serialization.

Sharding: 8 cores = 4 batches x 2 sequence halves.  Half 0 = windows 0..8
(rows 0:4736), half 1 = windows 9..18 (rows 3456:8192).  attn_mask is all
ones per the problem spec; a numpy fallback handles masks with zeros.

Scales: wk8+wkr ~ 8*Wk_core (copy 1/8), w28 ~ 4*W2 (copy 1/4),
qct = q^T/16, xh = X*HV/256 (lw = 8*dd/ss), gam64 = 64*Gamma,
z_out = 64*z in bf16 (host divides by 64).
"""
import os
import sys

for _p in ("/opt/trn_rl_repo", "/root/.axon_site/_ro/trn_rl_repo"):
    if os.path.isdir(_p) and _p not in sys.path:
        sys.path.insert(0, _p)

import numpy as np
import ml_dtypes

import concourse.bass as bass
import concourse.bass_isa as bass_isa
import concourse.mybir as mybir
import concourse.tile as tile
from concourse import bacc
from concourse.bass_utils import run_bass_kernel_spmd

F32 = mybir.dt.float32
E5D = mybir.dt.float8e5
BF16 = mybir.dt.bfloat16
FP8 = mybir.dt.float8e4
AF = mybir.ActivationFunctionType
ALU = mybir.AluOpType
DR = mybir.MatmulPerfMode.DoubleRow
E4 = ml_dtypes.float8_e4m3
E5 = ml_dtypes.float8_e5m2

B, L, D, T, DG, DP = 4, 8192, 1024, 512, 256, 256
WIN, STRIDE = 1536, 384
L_LOC, NCH, NCHP, NPAIR, NWIN = 4736, 37, 38, 19, 16
BLKS = [256, 256] + [512] * 7 + [384, 256]   # 4736 j-columns per core
BCG_SLABS = [6, 2, 6, 8, 8, 8]            # Gamma slab rows (sum 38); slab0
                                          # (chunks 0-5) via early Pool bcast


def _window_starts_eff():
    starts, s = [], 0
    while s < L:
        e = min(s + WIN, L)
        starts.append(min(s, L - WIN))   # jax dynamic_slice clamps
        if e == L:
            break
        s += STRIDE
    return starts


def _core_plan():
    starts = _window_starts_eff()
    assert len(starts) == 19
    halves = [dict(lo=0, wins=starts[0:9]), dict(lo=3456, wins=starts[9:19])]
    for h in halves:
        h["win_local"] = [(s - h["lo"]) // 128 for s in h["wins"]]
    return halves


def _build_bass(reps=1):
    nc = bacc.Bacc("TRN2", target_bir_lowering=False, debug=False)
    htb = nc.dram_tensor("htb", [2 * D, L_LOC], FP8, kind="ExternalInput")
    hn8 = nc.dram_tensor("hn8", [L_LOC, D], FP8, kind="ExternalInput")
    hnr = nc.dram_tensor("hnr", [L_LOC, D], FP8, kind="ExternalInput")
    qg8 = nc.dram_tensor("qg8", [DP + DG, T], FP8, kind="ExternalInput")
    qgr = nc.dram_tensor("qgr", [DP, T], E5D, kind="ExternalInput")
    wtk = nc.dram_tensor("wtk", [128, 16 * DP], FP8, kind="ExternalInput")
    wtw = nc.dram_tensor("wtw", [128, 8 * DP], FP8, kind="ExternalInput")
    winT = nc.dram_tensor("winT", [NWIN, NCHP], BF16, kind="ExternalInput")
    winrow8 = nc.dram_tensor("winrow8", [128, NPAIR * 2 * 128], FP8,
                             kind="ExternalInput")
    z_out = nc.dram_tensor("z_out", [T, D], BF16, kind="ExternalOutput")
    s_out = nc.dram_tensor("s_out", [NWIN, T], F32, kind="ExternalOutput")

    with tile.TileContext(nc) as tc:
        with (
            tc.tile_pool(name="dram", bufs=1, space="DRAM") as dpool,
            tc.tile_pool(name="const", bufs=1) as cpool,
            tc.tile_pool(name="res", bufs=1) as rpool,
        ):
            # ---- constants; sync queue carries the big H streams in
            # priority order, small consts ride the scalar queue
            warm_sb = cpool.tile([128, 256], BF16)
            nc.gpsimd.memset(warm_sb[:], 1.0)
            scale8_sb = cpool.tile([128, 1], F32)
            nc.gpsimd.memset(scale8_sb[:], 0.125)
            scale4_sb = cpool.tile([128, 1], F32)
            nc.gpsimd.memset(scale4_sb[:], 0.25)
            qg8_sb = cpool.tile([128, 4, T], FP8)
            qgr_sb = cpool.tile([128, 2, T], E5D)
            winrow8_sb = cpool.tile([128, NPAIR, 2, 128], FP8)
            winT_sb = cpool.tile([NWIN, NCHP], BF16)

            def emit_const_dmas(stage):
                # stage 0 after block0's H DMAs, stage 1 after block1's:
                # keeps the wtk/ht0 critical path first in the DMA queues
                if stage == 0:
                    nc.scalar.dma_start(
                        qg8_sb[:], qg8[:].rearrange("(c p) t -> p c t", p=128))
                    nc.scalar.dma_start(
                        qgr_sb[:], qgr[:].rearrange("(c p) t -> p c t", p=128))
                    nc.scalar.dma_start(
                        winrow8_sb[:, 0:3], winrow8[:, 0:3 * 256].rearrange(
                            "p (a b c) -> p a b c", b=2, c=128))
                else:
                    nc.scalar.dma_start(
                        winrow8_sb[:, 3:NPAIR],
                        winrow8[:, 3 * 256:].rearrange(
                            "p (a b c) -> p a b c", b=2, c=128))
                    nc.scalar.dma_start(winT_sb[:], winT[:])

            # ---- PE warmup on the locally-memset tile (no DMA dependency);
            # covers the ramp window until the first A matmul (~4us)
            with tc.tile_pool(name="warm", bufs=1, space="PSUM") as wps:
                wtile = wps.tile([128, 256], F32)
                for wi in range(12):
                    nc.tensor.matmul(wtile[:], warm_sb[:, 0:128],
                                     warm_sb[:], start=True, stop=True,
                                     skip_group_check=True)

            # ---- residents
            X_sb = rpool.tile([128, NCHP, T], BF16)     # [j%128, chunk, t]
            nc.gpsimd.memset(X_sb[:, NCH, :], 0.0)      # pad chunk 37
            hn8_sb = rpool.tile([128, NCHP, D], FP8)
            nc.gpsimd.memset(hn8_sb[:, NCH, :], 0.0)
            hnr_sb = rpool.tile([128, NCHP, D], FP8)
            nc.gpsimd.memset(hnr_sb[:, NCH, :], 0.0)
            # dedicated pre-zeroed tiles for the final (padded) chunk pair,
            # so no Pool memset sits on the end-of-phase critical path
            xh_pad = rpool.tile([128, 2, T], FP8)
            nc.gpsimd.memset(xh_pad[:, 1, :], 0.0)
            x8_pad = rpool.tile([128, 2, T], FP8)
            nc.gpsimd.memset(x8_pad[:, 1, :], 0.0)

            for _rep in range(reps):
                psAcc_cm = tc.tile_pool(name="psAcc", bufs=1, space="PSUM")
                psAcc = psAcc_cm.__enter__()
                # per-window column sums, accumulated per chunk-PAIR as fp8
                # DoubleRow matmuls: ss = sum X8 (winrow8 cols 0:64),
                # dd = sum X*HV (winrow8 cols 64:128), separate banks
                ss_acc = psAcc.tile([128, T], F32, tag="ssacc")
                dd_acc = psAcc.tile([128, T], F32, tag="ddacc")
                scpA_cm = tc.tile_pool(name="scA", bufs=1)
                scpA = scpA_cm.__enter__()
                with (
                    tc.tile_pool(name="a12", bufs=1) as apool,
                    tc.tile_pool(name="ht", bufs=3) as htpool,
                    tc.tile_pool(name="psA", bufs=2, space="PSUM") as psA,
                    tc.tile_pool(name="psS", bufs=2, space="PSUM") as psS,
                    tc.tile_pool(name="psHV", bufs=2, space="PSUM") as psHV,
                    tc.tile_pool(name="xh", bufs=4) as xhpool,
                    tc.tile_pool(name="x8", bufs=4) as x8pool,
                ):
                    A18_sb = apool.tile([128, 2, L_LOC], FP8, tag="A18")
                    A1r_sb = apool.tile([128, 2, L_LOC], E5D, tag="A1r")
                    A28_sb = apool.tile([128, 2, L_LOC], FP8, tag="A28")
                    # weight tiles live exactly as long as the A-phase;
                    # apool residency frees their SBUF for the z pools
                    wtk_sb = apool.tile([128, 16, DP], FP8, tag="wtk")
                    wtw_sb = apool.tile([128, 8, DP], FP8, tag="wtw")
                    xh_box = [None, None]
                    pair_box = [None, None, None]
                    bcgA = {}
                    pps = {}

                    def emit_ssdd(xh_t, x8_t, pr):
                        nc.tensor.matmul(
                            ss_acc[0:64, :], winrow8_sb[:, pr, :, 0:64],
                            x8_t[:],
                            start=(pr == 0), stop=(pr == NPAIR - 1),
                            perf_mode=DR, skip_group_check=True)
                        nc.tensor.matmul(
                            dd_acc[0:64, :], winrow8_sb[:, pr, :, 64:128],
                            xh_t[:],
                            start=(pr == 0), stop=(pr == NPAIR - 1),
                            perf_mode=DR, skip_group_check=True)
                        if pr == 7:
                            # windows 0/1 only collect from pairs 0..7:
                            # rows 0:2 of the accumulators are final now
                            # (later pairs add zero-weighted terms), so
                            # the early Gamma chain reads them mid-group
                            emit_chainA()

                    def emit_chainA():
                        # windows 0/1 are final: compute Gamma for chunks
                        # 0..5 now (bcg0 = w0, bcg3 = w0+w1), all on
                        # engines with in-phase slack (DVE/Act/Pool).
                        # bf16 is plenty: these only weight ~500-strong
                        # window softmax sums
                        with nc.allow_low_precision(
                                reason="bf16 Gamma weights"):
                            recA = scpA.tile([2, T], BF16)
                            nc.vector.reciprocal(recA[:], ss_acc[0:2, :])
                            lwA = scpA.tile([2, T], BF16)
                            nc.vector.scalar_tensor_tensor(
                                lwA[:], dd_acc[0:2, :], 0.5, recA[:],
                                op0=ALU.mult, op1=ALU.mult)
                            elwA = scpA.tile([2, T], BF16)
                            nc.scalar.activation(elwA[:], lwA[:], AF.Exp)
                            gamA = scpA.tile([2, T], BF16)
                            nc.vector.scalar_tensor_tensor(
                                gamA[:], elwA[:], 4.0, recA[:],
                                op0=ALU.mult, op1=ALU.mult)
                        bcg0 = scpA.tile([128, T], BF16)
                        nc.gpsimd.partition_broadcast(bcg0[:], gamA[0:1, :])
                        g01 = scpA.tile([2, T], BF16)
                        nc.gpsimd.partition_all_reduce(
                            g01[:], gamA[0:2, :], channels=2,
                            reduce_op=bass_isa.ReduceOp.add)
                        bcg3 = scpA.tile([128, T], BF16)
                        nc.gpsimd.partition_broadcast(bcg3[:], g01[0:1, :])
                        bcgA[0] = bcg0
                        # pr0's pp/pp8 as well: X chunks 0/1 are long
                        # final, and the mid-phase engines have slack, so
                        # the first z matmul gates only on PSUM release
                        pp_t0 = scpA.tile([128, 2, T], BF16)
                        nc.vector.tensor_mul(
                            pp_t0[:, 0, :], X_sb[:, 0, :], bcg0[:])
                        nc.vector.tensor_mul(
                            pp_t0[:, 1, :], X_sb[:, 1, :], bcg0[:])
                        pp8_t0 = scpA.tile([128, 2, T], FP8)
                        nc.scalar.activation(pp8_t0[:], pp_t0[:], AF.Copy)
                        pps[0] = (pp8_t0, pp_t0)
                        bcgA[1] = bcg3

                    def ph1_chunk(c):
                        if c == NCH - 1:
                            xh_box[0] = xh_pad
                            xh_box[1] = x8_pad
                        elif (c & 1) == 0:
                            xh_new = xhpool.tile([128, 2, T], FP8, tag="xh")
                            x8_new = x8pool.tile([128, 2, T], FP8, tag="x8")
                            xh_box[0] = xh_new
                            xh_box[1] = x8_new
                        xh_t, x8_t = xh_box
                        ps_s = psS.tile([128, T], F32, tag="psS")
                        cs = slice(c * 128, (c + 1) * 128)
                        for mm, (st, mv) in enumerate((
                            (A18_sb, qg8_sb[:, 0:2, :]),
                            (A1r_sb, qg8_sb[:, 0:2, :]),
                            (A18_sb, qgr_sb[:, 0:2, :]),
                        )):
                            nc.tensor.matmul(
                                ps_s[:], st[:, :, cs], mv,
                                start=(mm == 0), stop=(mm == 2),
                                perf_mode=DR, skip_group_check=True)
                        nc.scalar.activation(X_sb[:, c, :], ps_s[:], AF.Exp)
                        # fp8 image of X/16 for the ss window sums (Pool:
                        # Act/DVE have no headroom here; /16 keeps the
                        # largest exp(S) under e4m3's 448 ceiling)
                        nc.gpsimd.tensor_scalar_mul(
                            x8_t[:, c & 1, :], X_sb[:, c, :], 1.0 / 16.0)
                        ps_hv = psHV.tile([128, T], F32, tag="psHV")
                        nc.tensor.matmul(
                            ps_hv[:], A28_sb[:, :, cs], qg8_sb[:, 2:4, :],
                            perf_mode=DR, skip_group_check=True)
                        nc.vector.scalar_tensor_tensor(
                            xh_t[:, c & 1, :], X_sb[:, c, :], 1.0 / 256.0,
                            ps_hv[:], op0=ALU.mult, op1=ALU.mult)
                        if (c & 1) == 1 or c == NCH - 1:
                            pr = c // 2
                            # ss/dd matmuls run three pairs late: the Pool
                            # X8 casts and the early Gamma-chain reads
                            # never stall the PE stream
                            if pr > 2:
                                emit_ssdd(*pair_box[0])
                            pair_box[0] = pair_box[1]
                            pair_box[1] = pair_box[2]
                            pair_box[2] = (xh_t, x8_t, pr)

                    j0 = 0
                    for blk, jbw in enumerate(BLKS):
                        htt = htpool.tile([128, 16, 512], FP8, tag="ht")
                        if blk == 0:
                            nc.sync.dma_start(
                                wtk_sb[:, 0:8], wtk[:, 0:8 * DP].rearrange(
                                    "p (c m) -> p c m", m=DP))
                        nc.sync.dma_start(
                            htt[:, 0:8, :jbw],
                            htb[0:D, j0:j0 + jbw].rearrange(
                                "(c p) j -> p c j", p=128))
                        if blk == 0:
                            nc.sync.dma_start(
                                wtk_sb[:, 8:16], wtk[:, 8 * DP:].rearrange(
                                    "p (c m) -> p c m", m=DP))
                        nc.sync.dma_start(
                            htt[:, 8:16, :jbw],
                            htb[D:2 * D, j0:j0 + jbw].rearrange(
                                "(c p) j -> p c j", p=128))
                        if blk == 0:
                            nc.sync.dma_start(wtw_sb[:], wtw[:].rearrange(
                                "p (c m) -> p c m", m=DP))
                        if blk in (1, 3):
                            emit_const_dmas(0 if blk == 1 else 1)
                        # A1/A2 for this block: 3-pass fp8 DoubleRow
                        # wtk tile: wk8 at c 0:8, wkr at 8:16; wtw: w28
                        # ht tile: ht8 at 0, htr at 8
                        for (wsb, passes, is_a1) in (
                            (wtk_sb, ((0, 0), (8, 0), (0, 8)), True),
                            (wtw_sb, ((0, 0),), False),
                        ):
                            for pc in range(2):
                                ps = psA.tile([128, 512], F32, tag="psA")
                                nmm = 4 * len(passes)
                                mm = 0
                                for (wb, hb) in passes:
                                    for s in range(4):
                                        nc.tensor.matmul(
                                            ps[:, :jbw],
                                            wsb[:, wb + 2 * s:wb + 2 * s + 2,
                                                pc * 128:(pc + 1) * 128],
                                            htt[:, hb + 2 * s:hb + 2 * s + 2,
                                                :jbw],
                                            start=(mm == 0), stop=(mm == nmm - 1),
                                            perf_mode=DR, skip_group_check=True)
                                        mm += 1
                                if is_a1:
                                    nc.scalar.activation(
                                        A18_sb[:, pc, j0:j0 + jbw], ps[:, :jbw],
                                        AF.Copy, scale=scale8_sb[:])
                                    nc.vector.scalar_tensor_tensor(
                                        A1r_sb[:, pc, j0:j0 + jbw], ps[:, :jbw],
                                        0.125, A18_sb[:, pc, j0:j0 + jbw],
                                        op0=ALU.mult, op1=ALU.subtract)
                                else:
                                    nc.scalar.activation(
                                        A28_sb[:, pc, j0:j0 + jbw], ps[:, :jbw],
                                        AF.Copy, scale=scale4_sb[:])
                        # PH1 for the PREVIOUS block's chunks (their
                        # A copies completed during this block's matmuls)
                        if blk > 0:
                            c0 = (j0 - BLKS[blk - 1]) // 128
                            for ci in range(BLKS[blk - 1] // 128):
                                ph1_chunk(c0 + ci)
                        j0 += jbw
                    # natural-layout H loads queue behind the ht stream;
                    # hnr split so the early z pairs unblock first
                    nc.sync.dma_start(
                        hn8_sb[:, 0:NCH, :],
                        hn8[:].rearrange("(c p) d -> p c d", p=128))
                    nc.sync.dma_start(
                        hnr_sb[:, 0:20, :],
                        hnr[0:20 * 128, :].rearrange("(c p) d -> p c d", p=128))
                    nc.sync.dma_start(
                        hnr_sb[:, 20:NCH, :],
                        hnr[20 * 128:, :].rearrange("(c p) d -> p c d", p=128))
                    # PH1 flush for the last two blocks
                    for c in range((j0 - BLKS[-1]) // 128, NCH):
                        ph1_chunk(c)
                    emit_ssdd(*pair_box[0])   # final three pairs
                    emit_ssdd(*pair_box[1])
                    emit_ssdd(*pair_box[2])

                # ---- PH2: window scalars + Gamma slabs
                with (
                    tc.tile_pool(name="bcg", bufs=4) as bcgpool,
                    tc.tile_pool(name="pp", bufs=4) as pppool,
                    tc.tile_pool(name="pp8", bufs=6) as pp8pool,
                    tc.tile_pool(name="ppr", bufs=6) as pprpool,
                ):
                    scp_cm = tc.tile_pool(name="sc", bufs=1)
                    scp = scp_cm.__enter__()
                    gamc_cm = tc.tile_pool(name="gamc", bufs=3)
                    gamcpool = gamc_cm.__enter__()
                    if True:
                        # pr0's pp chain first (ready at flush end: bcg0
                        # was broadcast in-phase), then the scalar chain,
                        # then the rest — ordered so no queue blocks
                        bcgmap = (bcgA[0], bcgA[0], bcgA[0], bcgA[1],
                                  bcgA[1], bcgA[1])

                        def emit_pp(pr012):
                            pp_t = pppool.tile([128, 2, T], BF16, tag="pp")
                            nc.vector.tensor_mul(
                                pp_t[:, 0, :], X_sb[:, 2 * pr012, :],
                                bcgmap[2 * pr012][:])
                            nc.vector.tensor_mul(
                                pp_t[:, 1, :], X_sb[:, 2 * pr012 + 1, :],
                                bcgmap[2 * pr012 + 1][:])
                            pp8_t = pp8pool.tile([128, 2, T], FP8, tag="pp8")
                            nc.scalar.activation(pp8_t[:], pp_t[:], AF.Copy)
                            pps[pr012] = (pp8_t, pp_t)

                        def emit_ppr(pr012):
                            pp8x, ppx = pps[pr012]
                            ppr_t = pprpool.tile([128, 2, T], FP8, tag="ppr")
                            nc.vector.tensor_sub(ppr_t[:], ppx[:], pp8x[:])
                            pps[pr012] = (pp8x, ppr_t)

                        # ss here is sum(X)/16, so lw = 0.5*dd/ss16 and
                        # gam64 = 64*elw/(16*ss16) = 4*elw/ss16
                        rec_sb = scp.tile([NWIN, T], F32)
                        nc.vector.reciprocal(rec_sb[:], ss_acc[0:NWIN, :])
                        lw_sb = scp.tile([NWIN, T], F32)
                        nc.vector.scalar_tensor_tensor(
                            lw_sb[:], dd_acc[0:NWIN, :], 0.5, rec_sb[:],
                            op0=ALU.mult, op1=ALU.mult)
                        emit_ppr(0)
                        emit_pp(1)
                        elw_sb = scp.tile([NWIN, T], F32)
                        nc.scalar.activation(elw_sb[:], lw_sb[:], AF.Exp)
                        gam64 = scp.tile([NWIN, T], BF16)
                        nc.vector.scalar_tensor_tensor(
                            gam64[:], elw_sb[:], 4.0, rec_sb[:],
                            op0=ALU.mult, op1=ALU.mult)
                        nc.scalar.dma_start(s_out[:], elw_sb[:])
                        emit_ppr(1)
                        emit_pp(2)
                        emit_ppr(2)
                        gdram = dpool.tile([NCHP, T], BF16)
                        slab_rows = ((ss_acc, 32), (ss_acc, 64),
                                     (dd_acc, 32))

                        def emit_slab(k):
                            # slab matmuls rotate through spare 32-aligned
                            # rows of the ss/dd banks (no psW pool); they
                            # are emitted interleaved with the early z
                            # pairs so they never gate the z start
                            qn = BCG_SLABS[1 + k]
                            q0 = sum(BCG_SLABS[:1 + k])
                            acc, r0 = slab_rows[k % 3]
                            nc.tensor.matmul(
                                acc[r0:r0 + qn, :], winT_sb[:, q0:q0 + qn],
                                gam64[:], start=True, stop=True,
                                skip_group_check=True)
                            g16 = gamcpool.tile([8, T], BF16, tag="gamc")
                            if k % 2 == 0:
                                nc.vector.tensor_copy(g16[:qn, :],
                                                      acc[r0:r0 + qn, :])
                            else:
                                nc.scalar.activation(g16[:qn, :],
                                                     acc[r0:r0 + qn, :],
                                                     AF.Copy)
                            nc.scalar.dma_start(gdram[q0:q0 + qn, :],
                                                g16[:qn, :])

                    # ---- PH3: z = 64 * (X*Gamma)^T (Hfp8 + Hres), 3-pass DR
                    # tt0-2 accumulate in the 6 banks released by the A/X
                    # pools; tt3's 2 banks only exist after the last slab
                    # matmul releases psAcc, so tt3 runs a catch-up pass
                    with (
                        tc.tile_pool(name="zf", bufs=2) as zfpool,
                        tc.tile_pool(name="psZa", bufs=1,
                                     space="PSUM") as psZa,
                    ):
                        zps = [None] * 4
                        for tt in (0, 1, 2):
                            zp = psZa.tile([128, D], F32, tag=f"z{tt}")
                            zps[tt] = zp
                        z3h = [None, None]   # tt3 halves (reused ss/dd banks)

                        def zview(tt, dn):
                            if tt < 3:
                                return zps[tt][:, dn * 512:(dn + 1) * 512]
                            return z3h[dn][:, :]
                        slab_of = []
                        slab_base = []
                        b0 = 0
                        for k, qn in enumerate(BCG_SLABS):
                            slab_of += [k] * qn
                            slab_base.append(b0)
                            b0 += qn
                        bcg_tiles = [None] * len(BCG_SLABS)

                        def emit_bcast(k):
                            if (k < 1 or k >= len(BCG_SLABS)
                                    or bcg_tiles[k] is not None):
                                return
                            qn, b = BCG_SLABS[k], slab_base[k]
                            bt = bcgpool.tile([128, 8, T], BF16, tag="bcg")
                            nc.sync.dma_start(
                                bt[:, :qn, :],
                                gdram[b:b + qn, :][None, :, :].broadcast_to(
                                    [128, qn, T]))
                            bcg_tiles[k] = bt

                        NTAIL = 4        # last pairs run tt-outer so z
                        NCATCH = 5       # prs that predate the tt3 bank
                        # pass-outer per pr: all 8 (tt,dn) p8*h8 matmuls run
                        # before ppr/hnr are needed (hides the ppr latency)
                        ZPASS = ((0, 0), (0, 1), (1, 0))  # (use_ppr, use_hnr)
                        for pr in range(NPAIR):
                            c2 = 2 * pr
                            kb = slab_of[c2]
                            off = c2 - slab_base[kb]
                            if pr <= 2:
                                pp8_t, ppr_t = pps[pr]
                            else:
                                pp_t = pppool.tile([128, 2, T], BF16, tag="pp")
                                nc.vector.tensor_mul(
                                    pp_t[:], X_sb[:, c2:c2 + 2, :],
                                    bcg_tiles[kb][:, off:off + 2, :])
                                pp8_t = pp8pool.tile([128, 2, T], FP8,
                                                     tag="pp8")
                                nc.scalar.activation(pp8_t[:], pp_t[:],
                                                     AF.Copy)
                                ppr_t = pprpool.tile([128, 2, T], FP8,
                                                     tag="ppr")
                                nc.vector.tensor_sub(ppr_t[:], pp_t[:],
                                                     pp8_t[:])
                            pps[pr] = (pp8_t, ppr_t)
                            tts = range(3) if pr < NCATCH else range(4)
                            if pr >= NPAIR - NTAIL:
                                emit_bcast(kb + 1)
                                continue
                            for (upr, uhr) in ZPASS:
                                st = ppr_t if upr else pp8_t
                                mv = hnr_sb if uhr else hn8_sb
                                for tt in tts:
                                    for dn in range(2):
                                        nc.tensor.matmul(
                                            zview(tt, dn),
                                            st[:, :, tt * 128:(tt + 1) * 128],
                                            mv[:, c2:c2 + 2,
                                               dn * 512:(dn + 1) * 512],
                                            start=bool(pr == 0 and not upr
                                                       and not uhr),
                                            stop=False,
                                            perf_mode=DR,
                                            skip_group_check=True)
                            # slab k interleaves after pr k-1's matmuls;
                            # after the last slab, psAcc frees the final 2
                            # PSUM banks for tt3, which then catches up
                            if pr < len(BCG_SLABS) - 1:
                                emit_slab(pr)
                            emit_bcast(kb + 1)
                            if pr == NCATCH - 1:
                                z3a = psAcc.tile([128, T], F32, tag="ssacc")
                                z3b = psAcc.tile([128, T], F32, tag="ddacc")
                                z3h[0] = z3a
                                z3h[1] = z3b
                                for cpr in range(NCATCH):
                                    cp8, cpp = pps[cpr]
                                    cc2 = 2 * cpr
                                    for pi, (upr, uhr) in enumerate(ZPASS):
                                        st = cpp if upr else cp8
                                        mv = hnr_sb if uhr else hn8_sb
                                        for dn in range(2):
                                            nc.tensor.matmul(
                                                zview(3, dn),
                                                st[:, :, 384:512],
                                                mv[:, cc2:cc2 + 2,
                                                   dn * 512:(dn + 1) * 512],
                                                start=bool(cpr == 0
                                                           and pi == 0),
                                                stop=False,
                                                perf_mode=DR,
                                                skip_group_check=True)
                        for tt in range(4):
                            for pr in range(NPAIR - NTAIL, NPAIR):
                                c2 = 2 * pr
                                pp8_t, ppr_t = pps[pr]
                                lastp = pr == NPAIR - 1
                                for dn in range(2):
                                    for (upr, uhr) in ZPASS:
                                        st = ppr_t if upr else pp8_t
                                        mv = hnr_sb if uhr else hn8_sb
                                        nc.tensor.matmul(
                                            zview(tt, dn),
                                            st[:, :, tt * 128:(tt + 1) * 128],
                                            mv[:, c2:c2 + 2,
                                               dn * 512:(dn + 1) * 512],
                                            start=False,
                                            stop=bool(lastp and upr
                                                      and dn == 1),
                                            perf_mode=DR,
                                            skip_group_check=True)
                            if tt < 3:
                                zf = zfpool.tile([128, D], BF16, tag="zf")
                                if tt % 2 == 0:
                                    nc.vector.tensor_copy(zf[:], zps[tt][:])
                                else:
                                    nc.scalar.activation(zf[:], zps[tt][:],
                                                         AF.Copy)
                                zq = (nc.sync, nc.scalar, nc.gpsimd)[tt]
                                zq.dma_start(
                                    z_out[tt * 128:(tt + 1) * 128, :], zf[:])
                            else:
                                # final drain split in halves so the last
                                # copy+DMA chain is short
                                for dn in range(2):
                                    zf = zfpool.tile([128, 512], BF16,
                                                     tag="zfh")
                                    if dn == 0:
                                        nc.vector.tensor_copy(
                                            zf[:], z3h[0][:, :])
                                        nc.sync.dma_start(
                                            z_out[tt * 128:(tt + 1) * 128,
                                                  0:512], zf[:])
                                    else:
                                        nc.scalar.activation(
                                            zf[:], z3h[1][:, :], AF.Copy)
                                        nc.scalar.dma_start(
                                            z_out[tt * 128:(tt + 1) * 128,
                                                  512:1024], zf[:])
                    gamc_cm.__exit__(None, None, None)
                    scp_cm.__exit__(None, None, None)
                scpA_cm.__exit__(None, None, None)
                psAcc_cm.__exit__(None, None, None)
    nc.compile()
    return nc


_NC_CACHE = None


def _get_nc():
    global _NC_CACHE
    if _NC_CACHE is None:
        _NC_CACHE = _build_bass()
    return _NC_CACHE


def _numpy_fallback(H, G, attn_mask, Wq_core, Wk_core, Wq_win, Wk_win):
    """Reference semantics in numpy; used only if attn_mask has zeros."""
    starts = _window_starts_eff()
    q_t = G @ Wq_win
    scale = D ** -0.5
    out = np.zeros((B, T, D), np.float32)
    for b in range(B):
        m = np.full((T, 1), -np.inf, np.float32)
        ssum = np.zeros((T, 1), np.float32)
        z = np.zeros((T, D), np.float32)
        q = (G[b] @ Wq_core) / np.float32(DP ** 0.5)
        for s0 in starts:
            Hk = H[b, s0:s0 + WIN, :]
            mk = attn_mask[b, s0:s0 + WIN]
            k = Hk @ Wk_core
            sc = q @ k.T
            sc = np.where(mk[None, :], sc, np.float32(-1e30))
            sc -= sc.max(axis=-1, keepdims=True)
            al = np.exp(sc)
            al /= al.sum(axis=-1, keepdims=True)
            Zk = al @ Hk
            k_w = Zk @ Wk_win
            lw = (q_t[b] * k_w).sum(-1, keepdims=True) * scale
            m_new = np.maximum(m, lw)
            em, ew = np.exp(m - m_new), np.exp(lw - m_new)
            ssum = ssum * em + ew
            z = z * em + ew * Zk
            m = m_new
        out[b] = z / (ssum + 1e-8)
    return out


def _fp8_split(x):
    """x (f32) -> (fp8 main, fp8 residual); main+res reconstructs x closely."""
    m = x.astype(E4)
    r = (x - m.astype(np.float32)).astype(E4)
    return m, r


def _pack_pmajor(w):
    """[c*128, m] fp8 -> [128, c*m] partition-major (full-BW DMA lines)."""
    cm, m = w.shape
    c = cm // 128
    return np.ascontiguousarray(
        w.reshape(c, 128, m).transpose(1, 0, 2).reshape(128, c * m))


def kernel(H, G, attn_mask, Wq_core, Wk_core, Wq_win, Wk_win):
    H = np.asarray(H, np.float32)
    G = np.asarray(G, np.float32)
    Wq_core = np.asarray(Wq_core, np.float32)
    Wk_core = np.asarray(Wk_core, np.float32)
    Wq_win = np.asarray(Wq_win, np.float32)
    Wk_win = np.asarray(Wk_win, np.float32)
    mask = np.asarray(attn_mask)
    if not mask.all():
        return _numpy_fallback(H, G, mask, Wq_core, Wk_core, Wq_win, Wk_win)

    halves = _core_plan()
    bf = ml_dtypes.bfloat16
    wk8_h, wkr_h = _fp8_split(8.0 * Wk_core)
    w2 = Wk_win @ Wq_win.T                                  # [D, DG]
    w28_h, _ = _fp8_split(4.0 * w2)
    wtk_h = _pack_pmajor(np.concatenate([wk8_h, wkr_h], axis=0))
    wtw_h = _pack_pmajor(np.ascontiguousarray(w28_h))

    in_maps = []
    for b in range(B):
        q_coreT = np.ascontiguousarray((G[b] @ Wq_core).T / 16.0)
        q8_h = q_coreT.astype(E4)
        qr_h = np.ascontiguousarray(
            (q_coreT - q8_h.astype(np.float32)).astype(E5))
        g8_h = np.ascontiguousarray(G[b].T).astype(E4)
        for h in halves:
            wloc = h["win_local"]
            nwin = len(wloc)
            win = np.zeros((NCHP, NWIN), np.float32)
            for w, cw in enumerate(wloc):
                win[cw:cw + 12, w] = 1.0
            winT = np.ascontiguousarray(win.T)   # dummy cols all zero
            # dummy window columns get a harmless nonzero row so the window
            # sum E stays finite; winT zeros keep them out of Gamma, and the
            # host ignores their s_out rows.
            win[NCH - 1, nwin:] = 1.0
            # winrow8 per (pair, chunk-of-pair): cols 0:64 drive the ss
            # accumulator, cols 64:128 the dd accumulator (same weights)
            winrow8 = np.zeros((128, NPAIR * 2 * 128), np.float32)
            for pr in range(NPAIR):
                for i in range(2):
                    base = pr * 256 + i * 128
                    winrow8[:, base:base + NWIN] = win[2 * pr + i]
                    winrow8[:, base + 64:base + 64 + NWIN] = win[2 * pr + i]
            Hs = np.ascontiguousarray(H[b, h["lo"]:h["lo"] + L_LOC, :])
            hn8_h, hnr_h = _fp8_split(Hs)
            in_maps.append(dict(
                htb=np.ascontiguousarray(np.concatenate(
                    [hn8_h.T, hnr_h.T], axis=0)),
                hn8=hn8_h, hnr=hnr_h,
                qg8=np.ascontiguousarray(np.concatenate([q8_h, g8_h], axis=0)),
                qgr=qr_h,
                wtk=wtk_h, wtw=wtw_h,
                winT=winT.astype(bf), winrow8=winrow8.astype(E4)))

    global _last_in_maps
    _last_in_maps = in_maps
    nc = _get_nc()
    res = run_bass_kernel_spmd(nc, in_maps, core_ids=list(range(8)))
    out = np.zeros((B, T, D), np.float32)
    nw0 = len(halves[0]["win_local"])
    nw1 = len(halves[1]["win_local"])
    for b in range(B):
        r0, r1 = res.results[2 * b], res.results[2 * b + 1]
        denom = (r0["s_out"][:nw0].sum(axis=0) + r1["s_out"][:nw1].sum(axis=0)
                 + 1e-8)
        out[b] = (r0["z_out"].astype(np.float32)
                  + r1["z_out"].astype(np.float32)) / 64.0 / denom[:, None]
    return out
